# revision 7
# baseline (speedup 1.0000x reference)
"""GPTQ 4-bit quantized linear (CaiQuantLinear) on 8 TRN2 NeuronCores.

Computes out = x @ dequant(qweight, scales, qzeros) + bias where
  x: (4, 2048, 4096) fp16, qweight: (512, 4096) int32 (8x 4-bit per word,
  packed along input features), scales: (32, 4096) fp16, qzeros: (32, 512)
  int32 (packed along output features), bias: (4096,) fp16.
  Groups are contiguous blocks of 128 input features (g_idx = arange//128).

Sharding: 2x4 grid — 2 sequence-row groups x 4 output-column groups.
Each core gets 4096 seq rows (half of x) and 1024 output columns. This
halves the per-core x DMA-transpose volume (the original bottleneck: the
single HWDGE xbar ring runs at ~150-180 GB/s and starved the PE) while
the dequant work, which runs on otherwise-idle engines, only doubles.
No collectives; the host assembles the 2x4 output grid.

Dequant with ZERO tensor-engine work: qweight is unpacked in its natural
[word-row, out] layout. Nibble s of word-row wr is input feature
k = 8*wr + s, so nibble-plane s of a 128-row word tile is a [128, out]
tile whose partition p holds k = 8*(128r+p) + s. That is a valid matmul
rhs k-tile under a fixed permutation of k — and the contraction is
permutation-invariant as long as lhsT (x^T) rows use the SAME
permutation. The host uploads x with its columns pre-ordered into
(r, s, wr) blocks so DMA-transposed 128-col blocks of x line up with the
nibble planes exactly. Scales/zeros are uploaded with group rows
pre-expanded 16x ([128, ...] layout) so dequant is three flat fp16
element-wise passes per k-tile (unpack, subtract z+1, scale), split
50/50 between the vector and gpsimd engines straight into the resident
weight buffer.

Engine placement (all DMA off the SWDGE path so the framework never
serializes it against the xbar transposes): input loads + output stores
on the scalar/ACT HWDGE queue; x transposes on the sync/SP HWDGE queue;
psum drains (bias add fp32->fp16) on vector. Main loop: 4 chunks of 1024
seq rows; 32 transposes ([1024,128] -> [128,1024], 256 KB) per chunk
feed k-major matmuls; psum tiles are [128, 1024] fp32 (2 banks). Chunk 0
runs its seq tiles in groups of 4 (8 matmuls per k-tile) so the PE's
k-consumption pace matches the dequant production pace; later chunks use
groups of 2 so drains pipeline. The PE does nothing but the 2048 N=512
matmuls.
"""

import sys

if "/opt/trn_rl_repo" not in sys.path:
    sys.path.insert(0, "/opt/trn_rl_repo")

import numpy as np

B, S, IN, OUT = 4, 2048, 4096, 4096
SEQ = B * S                      # 8192
NCORES = 8
RGRP, CGRP = 2, 4                # core grid: 2 row groups x 4 col groups
SEQ_S = SEQ // RGRP              # 4096 seq rows per core
OUT_S = OUT // CGRP              # 1024 output columns per core
PACK = 8                         # int32 packs 8 nibbles
GSIZE = 128                      # group size == k-tile size
CHUNK = 1024                     # seq rows per transpose chunk
RBLK = IN // (PACK * 128)        # 4 word-row blocks of 128

_CACHE = {}


def _build(seq, out_s, chunk):
    """Build + compile the per-core Bass program. All cores run the same
    NEFF on their own input slices (SPMD, no collectives)."""
    import concourse.bass as bass  # noqa: F401
    import concourse.mybir as mybir
    import concourse.tile as tile
    from concourse import bacc

    dt = mybir.dt
    op = mybir.AluOpType
    P = 128
    KT = IN // P                  # 32 k-tiles (= groups), order (r, s)
    QR = IN // PACK               # 512 qweight word rows
    NCH = seq // chunk            # 4 seq chunks
    ST = chunk // P               # 8 seq tiles per chunk
    NB = out_s // 512             # 2 matmul n-blocks per psum tile
    ZC = out_s // PACK            # 128 qzeros word cols per core

    nc = bacc.Bacc("TRN2", target_bir_lowering=False, debug=False,
                   num_devices=NCORES)

    x_d = nc.dram_tensor("x", (seq, IN), dt.float16, kind="ExternalInput")
    qw_d = nc.dram_tensor("qweight", (QR, out_s), dt.int32,
                          kind="ExternalInput")
    se_d = nc.dram_tensor("sexp", (P, RBLK * out_s), dt.float16,
                          kind="ExternalInput")
    qz_d = nc.dram_tensor("qzexp", (P, RBLK * ZC), dt.int32,
                          kind="ExternalInput")
    b_d = nc.dram_tensor("bias", (1, out_s), dt.float16, kind="ExternalInput")
    out_d = nc.dram_tensor("out", (seq, out_s), dt.float16,
                           kind="ExternalOutput")

    x = x_d.ap()
    qw = qw_d.ap()
    sexp_in = se_d.ap()
    qzexp_in = qz_d.ap()
    bias = b_d.ap()
    out = out_d.ap()

    with tile.TileContext(nc) as tc:
        with (
            tc.tile_pool(name="const", bufs=1) as const_pool,
            tc.tile_pool(name="w", bufs=1) as w_pool,
            tc.tile_pool(name="qn", bufs=2) as qn_pool,
            tc.tile_pool(name="wq", bufs=2) as wq_pool,
            tc.tile_pool(name="wd", bufs=3) as wd_pool,
            tc.tile_pool(name="xt", bufs=39) as xt_pool,
            tc.tile_pool(name="ot", bufs=4) as out_pool,
            tc.tile_pool(name="ps", bufs=4, space="PSUM") as psum_pool,
        ):
            # ---- input loads: scalar/ACT HWDGE queue, deps-first order ----
            qz_exp = const_pool.tile([P, RBLK * ZC], dt.int32)
            nc.scalar.dma_start(qz_exp, qzexp_in)
            q_nats = []
            for r in range(RBLK):
                q_nat = qn_pool.tile([P, out_s], dt.int32, tag="qn")
                nc.scalar.dma_start(q_nat, qw[r * P:(r + 1) * P, :])
                q_nats.append(q_nat)
            s_exp = const_pool.tile([P, RBLK * out_s], dt.float16)
            nc.scalar.dma_start(s_exp, sexp_in)
            bias16 = const_pool.tile([P, out_s], dt.float16)
            nc.scalar.dma_start(bias16, bias.to_broadcast((P, out_s)))
            bias32 = const_pool.tile([P, out_s], dt.float32)
            nc.vector.tensor_copy(bias32, bias16)

            # z+1 in expanded [P, (r, n)] layout, int32 (bitVec ops can't
            # cast on write, so the unpack stays int; the arithmetic
            # subtract below casts to fp16)
            z1 = const_pool.tile([P, RBLK * out_s], dt.int32)
            z1v = z1.rearrange("p (r c s) -> p r c s", r=RBLK, s=PACK)
            qzv = qz_exp.rearrange("p (r c) -> p r c", r=RBLK)
            for sz in range(PACK):
                nc.vector.tensor_scalar(
                    out=z1v[:, :, :, sz], in0=qzv, scalar1=4 * sz,
                    scalar2=0xF, op0=op.logical_shift_right,
                    op1=op.bitwise_and)
            nc.vector.tensor_scalar_add(z1, z1, 1)

            # ---- dequant: 3 passes per k-tile, split across engines ----
            # vector: unpack (bitVec, int32) + subtract (int32 -> fp16 cast);
            # gpsimd: the pure-fp16 multiply by scales into w_all. This
            # pipelines the passes across two engines (~1.9 us/k-tile on
            # vector instead of ~3.1 all-vector).
            # w_all[:, j, :]: k-tile j=(r, s); partition p holds input
            # feature k = 8*(128 r + p) + s.
            w_all = w_pool.tile([P, KT, out_s], dt.float16)

            for j in range(KT):
                r, s = j // PACK, j % PACK
                wq = wq_pool.tile([P, out_s], dt.int32, tag="wq")
                nc.vector.tensor_scalar(
                    out=wq, in0=q_nats[r], scalar1=4 * s, scalar2=0xF,
                    op0=op.logical_shift_right, op1=op.bitwise_and)
                wd = wd_pool.tile([P, out_s], dt.float16, tag="wd")
                nc.vector.tensor_tensor(
                    wd, wq, z1[:, r * out_s:(r + 1) * out_s], op.subtract)
                nc.gpsimd.tensor_tensor(
                    w_all[:, j, :], wd, s_exp[:, r * out_s:(r + 1) * out_s],
                    op.mult)

            # ---- main loop ----
            # psum[m, n] = sum_j xT_j[p, m] * w_j[p, n]; both sides use the
            # same k permutation, so this equals the natural contraction.
            def do_chunk(ch, sg_size):
                xts = []
                for j in range(KT):
                    xtk = xt_pool.tile([P, chunk], dt.float16, tag="xt")
                    nc.sync.dma_start(
                        xtk,
                        x[ch * chunk:(ch + 1) * chunk, j * P:(j + 1) * P],
                        transpose=True)
                    xts.append(xtk)
                for sg in range(ST // sg_size):
                    sts = range(sg * sg_size, (sg + 1) * sg_size)
                    pss = [psum_pool.tile([P, out_s], dt.float32, tag="acc",
                                          name=f"ps_{ch}_{st}")
                           for st in sts]
                    for j in range(KT):
                        for i, st in enumerate(sts):
                            for nb in range(NB):
                                nc.tensor.matmul(
                                    pss[i][:, nb * 512:(nb + 1) * 512],
                                    lhsT=xts[j][:, st * P:(st + 1) * P],
                                    rhs=w_all[:, j, nb * 512:(nb + 1) * 512],
                                    start=(j == 0), stop=(j == KT - 1))
                    for i, st in enumerate(sts):
                        o16 = out_pool.tile([P, out_s], dt.float16,
                                            tag="o16")
                        nc.vector.tensor_add(o16, pss[i], bias32)
                        r0 = ch * chunk + st * P
                        nc.scalar.dma_start(out[r0:r0 + P, :], o16)

            # chunk 0: 8 matmuls per k-tile so PE pace matches dequant pace
            do_chunk(0, 4)
            for ch in range(1, NCH):
                do_chunk(ch, 2)

    nc.compile()
    return nc


def _get_program():
    key = (SEQ_S, OUT_S, CHUNK)
    if key not in _CACHE:
        _CACHE[key] = _build(SEQ_S, OUT_S, CHUNK)
    return _CACHE[key]


def _permute_x(x):
    """Reorder x columns into (r, s, wr) blocks: new col (r*8+s)*128 + w
    holds old input feature k = 8*(128 r + w) + s."""
    x2 = np.asarray(x).reshape(SEQ, IN)
    return np.ascontiguousarray(
        x2.reshape(SEQ, RBLK, 128, PACK).transpose(0, 1, 3, 2)
        .reshape(SEQ, IN))


def _expand_groups(a):
    """[32, n] per-group rows -> [128, 4*n]: out[p, r*n + j] =
    a[8*r + p//16, j] (matches the group of k = 8*(128 r + p) + s)."""
    n = a.shape[1]
    e = np.broadcast_to(a.reshape(RBLK, PACK, 1, n), (RBLK, PACK, 16, n))
    return np.ascontiguousarray(
        e.transpose(1, 2, 0, 3).reshape(128, RBLK * n))


def shard_inputs(x, qweight, scales, qzeros, bias):
    """Build the 8 per-core input maps for run_bass_kernel_spmd."""
    xp = _permute_x(x)
    qweight = np.asarray(qweight)
    scales = np.asarray(scales)
    qzeros = np.asarray(qzeros)
    bias = np.asarray(bias)

    in_maps = []
    for c in range(NCORES):
        rg, cg = c // CGRP, c % CGRP
        o0 = cg * OUT_S
        in_maps.append({
            "x": xp[rg * SEQ_S:(rg + 1) * SEQ_S],
            "qweight": np.ascontiguousarray(qweight[:, o0:o0 + OUT_S]),
            "sexp": _expand_groups(scales[:, o0:o0 + OUT_S]),
            "qzexp": _expand_groups(
                qzeros[:, cg * (OUT_S // PACK):(cg + 1) * (OUT_S // PACK)]),
            "bias": np.ascontiguousarray(bias[o0:o0 + OUT_S].reshape(1, -1)),
        })
    return in_maps


def assemble_output(results):
    """Stitch the 2x4 per-core output grid into the full (B, S, OUT)."""
    full = np.empty((SEQ, OUT), dtype=np.float16)
    for c in range(NCORES):
        rg, cg = c // CGRP, c % CGRP
        full[rg * SEQ_S:(rg + 1) * SEQ_S,
             cg * OUT_S:(cg + 1) * OUT_S] = results[c]["out"]
    return full.reshape(B, S, OUT)


def kernel(x, qweight, scales, qzeros, g_idx=None, bias=None, **_unused):
    """Full-input entry point: shards over 8 cores, runs on HW, gathers."""
    from concourse.bass_utils import run_bass_kernel_spmd

    nc = _get_program()
    in_maps = shard_inputs(x, qweight, scales, qzeros, bias)
    res = run_bass_kernel_spmd(nc, in_maps, core_ids=list(range(NCORES)))
    return assemble_output(res.results)


# revision 8
# speedup vs baseline: 1.1172x; 1.1172x over previous
"""GPTQ 4-bit quantized linear (CaiQuantLinear) on 8 TRN2 NeuronCores.

Computes out = x @ dequant(qweight, scales, qzeros) + bias where
  x: (4, 2048, 4096) fp16, qweight: (512, 4096) int32 (8x 4-bit per word,
  packed along input features), scales: (32, 4096) fp16, qzeros: (32, 512)
  int32 (packed along output features), bias: (4096,) fp16.
  Groups are contiguous blocks of 128 input features (g_idx = arange//128).

Sharding: 2x4 grid — 2 sequence-row groups x 4 output-column groups.
Each core gets 4096 seq rows (half of x) and 1024 output columns. This
halves the per-core x DMA-transpose volume (the original bottleneck: the
single HWDGE xbar ring runs at ~150-180 GB/s and starved the PE) while
the dequant work, which runs on the otherwise-idle vector engine, only
doubles. No collectives; the host assembles the 2x4 output grid.

Dequant with ZERO tensor-engine work: qweight is unpacked in its natural
[word-row, out] layout. Nibble s of word-row wr is input feature
k = 8*wr + s, so nibble-plane s of a 128-row word tile is a [128, out]
tile whose partition p holds k = 8*(128r+p) + s. That is a valid matmul
rhs k-tile under a fixed permutation of k — and the contraction is
permutation-invariant as long as lhsT (x^T) rows use the SAME
permutation. The host uploads x with its columns pre-ordered into
(r, s, wr) blocks so DMA-transposed 128-col blocks of x line up with the
nibble planes exactly. Scales/zeros are uploaded with group rows
pre-expanded 16x ([128, ...] layout) so dequant is three flat vector
passes per k-tile (unpack, subtract z+1 with int32->fp16 cast, scale).

Scheduling notes (hard-won from traces):
- Tile serializes every DMA-transpose against every other in-flight DMA
  on any queue (HW deadlock guard), with multi-us round trips. So the
  input loads are issued on the SAME sync queue AHEAD of the transposes
  (strict FIFO -> they complete first, no ping-pong), and only the
  infrequent output stores (scalar/ACT queue) interleave mid-run.
- gpsimd tensor ops slow concurrent vector ops ~3x (SBUF port
  contention) — dequant stays vector-only.
- The vector engine produces dequantized weights at ~1.55 us per
  [128,512] half-tile while the PE consumes k-tiles at 1.73 us (8
  matmuls, the 8-bank psum cap). Dequant emits LEFT halves of all 32
  k-tiles, then RIGHT halves; chunk 0 does two full-chunk k-sweeps (left
  cols for all 8 seq tiles, then right cols), so the PE never outruns
  dequant. Later chunks use 2-seq-tile groups so drains pipeline.
Main loop: 4 chunks of 1024 seq rows; 32 transposes ([1024,128] ->
[128,1024], 256 KB) per chunk; psum tiles are [128, 512] fp32 (1 bank);
drains add bias fp32->fp16 on vector; stores via scalar/ACT HWDGE. The
PE does nothing but the 4096 N=512 matmuls.
"""

import sys

if "/opt/trn_rl_repo" not in sys.path:
    sys.path.insert(0, "/opt/trn_rl_repo")

import numpy as np

B, S, IN, OUT = 4, 2048, 4096, 4096
SEQ = B * S                      # 8192
NCORES = 8
RGRP, CGRP = 2, 4                # core grid: 2 row groups x 4 col groups
SEQ_S = SEQ // RGRP              # 4096 seq rows per core
OUT_S = OUT // CGRP              # 1024 output columns per core
PACK = 8                         # int32 packs 8 nibbles
GSIZE = 128                      # group size == k-tile size
CHUNK = 1024                     # seq rows per transpose chunk
RBLK = IN // (PACK * 128)        # 4 word-row blocks of 128

_CACHE = {}


def _build(seq, out_s, chunk):
    """Build + compile the per-core Bass program. All cores run the same
    NEFF on their own input slices (SPMD, no collectives)."""
    import concourse.bass as bass  # noqa: F401
    import concourse.mybir as mybir
    import concourse.tile as tile
    from concourse import bacc

    dt = mybir.dt
    op = mybir.AluOpType
    P = 128
    KT = IN // P                  # 32 k-tiles (= groups), order (r, s)
    QR = IN // PACK               # 512 qweight word rows
    NCH = seq // chunk            # 4 seq chunks
    ST = chunk // P               # 8 seq tiles per chunk
    NB = out_s // 512             # 2 psum n-blocks (halves) per seq tile
    ZC = out_s // PACK            # 128 qzeros word cols per core
    H = 512                       # psum half width

    nc = bacc.Bacc("TRN2", target_bir_lowering=False, debug=False,
                   num_devices=NCORES)

    x_d = nc.dram_tensor("x", (seq, IN), dt.float16, kind="ExternalInput")
    qw_d = nc.dram_tensor("qweight", (QR, out_s), dt.int32,
                          kind="ExternalInput")
    se_d = nc.dram_tensor("sexp", (P, RBLK * out_s), dt.float16,
                          kind="ExternalInput")
    qz_d = nc.dram_tensor("qzexp", (P, RBLK * ZC), dt.int32,
                          kind="ExternalInput")
    b_d = nc.dram_tensor("bias", (1, out_s), dt.float16, kind="ExternalInput")
    out_d = nc.dram_tensor("out", (seq, out_s), dt.float16,
                           kind="ExternalOutput")

    x = x_d.ap()
    qw = qw_d.ap()
    sexp_in = se_d.ap()
    qzexp_in = qz_d.ap()
    bias = b_d.ap()
    out = out_d.ap()

    with tile.TileContext(nc) as tc:
        with (
            tc.tile_pool(name="const", bufs=1) as const_pool,
            tc.tile_pool(name="w", bufs=1) as w_pool,
            tc.tile_pool(name="qn", bufs=RBLK) as qn_pool,
            tc.tile_pool(name="wq", bufs=2) as wq_pool,
            tc.tile_pool(name="wd", bufs=2) as wd_pool,
            tc.tile_pool(name="xt", bufs=39) as xt_pool,
            tc.tile_pool(name="ot", bufs=4) as out_pool,
            tc.tile_pool(name="ps", bufs=8, space="PSUM") as psum_pool,
        ):
            # ---- input loads: on the SYNC queue, ahead of all transposes
            # (strict FIFO avoids the transpose-vs-DMA serialization) ----
            qz_exp = const_pool.tile([P, RBLK * ZC], dt.int32)
            nc.sync.dma_start(qz_exp, qzexp_in)
            q_nats = []
            for r in range(RBLK):
                q_nat = qn_pool.tile([P, out_s], dt.int32, tag="qn")
                nc.sync.dma_start(q_nat, qw[r * P:(r + 1) * P, :])
                q_nats.append(q_nat)
            s_exp = const_pool.tile([P, RBLK * out_s], dt.float16)
            nc.sync.dma_start(s_exp, sexp_in)
            # bias: scalar queue (off the critical path, needed at drains)
            bias16 = const_pool.tile([P, out_s], dt.float16)
            nc.scalar.dma_start(bias16, bias.to_broadcast((P, out_s)))
            bias32 = const_pool.tile([P, out_s], dt.float32)
            nc.vector.tensor_copy(bias32, bias16)

            # z+1 in expanded [P, (r, n)] layout, int32 (bitVec unpack
            # can't cast; the arithmetic subtract below casts to fp16)
            z1 = const_pool.tile([P, RBLK * out_s], dt.int32)
            z1v = z1.rearrange("p (r c s) -> p r c s", r=RBLK, s=PACK)
            qzv = qz_exp.rearrange("p (r c) -> p r c", r=RBLK)
            for sz in range(PACK):
                nc.vector.tensor_scalar(
                    out=z1v[:, :, :, sz], in0=qzv, scalar1=4 * sz,
                    scalar2=0xF, op0=op.logical_shift_right,
                    op1=op.bitwise_and)
            nc.vector.tensor_scalar_add(z1, z1, 1)

            # ---- dequant (vector-only), by (half, k-tile) ----
            # w_all[:, j, :]: k-tile j=(r, s); partition p holds input
            # feature k = 8*(128 r + p) + s.
            w_all = w_pool.tile([P, KT, out_s], dt.float16)

            def dequant_half(j, nb):
                r, s = j // PACK, j % PACK
                c0 = nb * H
                wq = wq_pool.tile([P, H], dt.int32, tag="wq")
                nc.vector.tensor_scalar(
                    out=wq, in0=q_nats[r][:, c0:c0 + H], scalar1=4 * s,
                    scalar2=0xF, op0=op.logical_shift_right,
                    op1=op.bitwise_and)
                wd = wd_pool.tile([P, H], dt.float16, tag="wd")
                nc.vector.tensor_tensor(
                    wd, wq, z1[:, r * out_s + c0:r * out_s + c0 + H],
                    op.subtract)
                nc.vector.tensor_tensor(
                    w_all[:, j, c0:c0 + H], wd,
                    s_exp[:, r * out_s + c0:r * out_s + c0 + H], op.mult)

            def drain(pss_st_nb, ch, sts):
                # bias-add psum halves into fp16 out tiles, store rows
                for st in sts:
                    o16 = out_pool.tile([P, out_s], dt.float16, tag="o16")
                    for nb in range(NB):
                        nc.vector.tensor_add(
                            o16[:, nb * H:(nb + 1) * H], pss_st_nb[st][nb],
                            bias32[:, nb * H:(nb + 1) * H])
                    r0 = ch * chunk + st * P
                    nc.scalar.dma_start(out[r0:r0 + P, :], o16)

            # ---- chunk 0: transposes + left-half dequant + left k-sweep
            # over all 8 seq tiles, then right halves ----
            xts0 = []
            for j in range(KT):
                xtk = xt_pool.tile([P, chunk], dt.float16, tag="xt")
                nc.sync.dma_start(
                    xtk, x[0:chunk, j * P:(j + 1) * P], transpose=True)
                xts0.append(xtk)

            ps0 = {st: [None, None] for st in range(ST)}
            for j in range(KT):
                dequant_half(j, 0)
                for st in range(ST):
                    if j == 0:
                        ps0[st][0] = psum_pool.tile(
                            [P, H], dt.float32, tag="acc", name=f"psA_{st}")
                    nc.tensor.matmul(
                        ps0[st][0], lhsT=xts0[j][:, st * P:(st + 1) * P],
                        rhs=w_all[:, j, 0:H],
                        start=(j == 0), stop=(j == KT - 1))
            for j in range(KT):
                dequant_half(j, 1)
                for st in range(ST):
                    if j == 0:
                        ps0[st][1] = psum_pool.tile(
                            [P, H], dt.float32, tag="acc", name=f"psB_{st}")
                    nc.tensor.matmul(
                        ps0[st][1], lhsT=xts0[j][:, st * P:(st + 1) * P],
                        rhs=w_all[:, j, H:out_s],
                        start=(j == 0), stop=(j == KT - 1))
            drain(ps0, 0, range(ST))

            # ---- chunks 1..3: 2-seq-tile groups, drains pipeline ----
            for ch in range(1, NCH):
                xts = []
                for j in range(KT):
                    xtk = xt_pool.tile([P, chunk], dt.float16, tag="xt")
                    nc.sync.dma_start(
                        xtk,
                        x[ch * chunk:(ch + 1) * chunk, j * P:(j + 1) * P],
                        transpose=True)
                    xts.append(xtk)
                for sg in range(ST // 2):
                    sts = (2 * sg, 2 * sg + 1)
                    pss = {st: [psum_pool.tile([P, H], dt.float32,
                                               tag="acc",
                                               name=f"ps_{ch}_{st}_{nb}")
                                for nb in range(NB)] for st in sts}
                    for j in range(KT):
                        for st in sts:
                            for nb in range(NB):
                                nc.tensor.matmul(
                                    pss[st][nb],
                                    lhsT=xts[j][:, st * P:(st + 1) * P],
                                    rhs=w_all[:, j, nb * H:(nb + 1) * H],
                                    start=(j == 0), stop=(j == KT - 1))
                    drain(pss, ch, sts)

    nc.compile()
    return nc


def _get_program():
    key = (SEQ_S, OUT_S, CHUNK)
    if key not in _CACHE:
        _CACHE[key] = _build(SEQ_S, OUT_S, CHUNK)
    return _CACHE[key]


def _permute_x(x):
    """Reorder x columns into (r, s, wr) blocks: new col (r*8+s)*128 + w
    holds old input feature k = 8*(128 r + w) + s."""
    x2 = np.asarray(x).reshape(SEQ, IN)
    return np.ascontiguousarray(
        x2.reshape(SEQ, RBLK, 128, PACK).transpose(0, 1, 3, 2)
        .reshape(SEQ, IN))


def _expand_groups(a):
    """[32, n] per-group rows -> [128, 4*n]: out[p, r*n + j] =
    a[8*r + p//16, j] (matches the group of k = 8*(128 r + p) + s)."""
    n = a.shape[1]
    e = np.broadcast_to(a.reshape(RBLK, PACK, 1, n), (RBLK, PACK, 16, n))
    return np.ascontiguousarray(
        e.transpose(1, 2, 0, 3).reshape(128, RBLK * n))


def shard_inputs(x, qweight, scales, qzeros, bias):
    """Build the 8 per-core input maps for run_bass_kernel_spmd."""
    xp = _permute_x(x)
    qweight = np.asarray(qweight)
    scales = np.asarray(scales)
    qzeros = np.asarray(qzeros)
    bias = np.asarray(bias)

    in_maps = []
    for c in range(NCORES):
        rg, cg = c // CGRP, c % CGRP
        o0 = cg * OUT_S
        in_maps.append({
            "x": xp[rg * SEQ_S:(rg + 1) * SEQ_S],
            "qweight": np.ascontiguousarray(qweight[:, o0:o0 + OUT_S]),
            "sexp": _expand_groups(scales[:, o0:o0 + OUT_S]),
            "qzexp": _expand_groups(
                qzeros[:, cg * (OUT_S // PACK):(cg + 1) * (OUT_S // PACK)]),
            "bias": np.ascontiguousarray(bias[o0:o0 + OUT_S].reshape(1, -1)),
        })
    return in_maps


def assemble_output(results):
    """Stitch the 2x4 per-core output grid into the full (B, S, OUT)."""
    full = np.empty((SEQ, OUT), dtype=np.float16)
    for c in range(NCORES):
        rg, cg = c // CGRP, c % CGRP
        full[rg * SEQ_S:(rg + 1) * SEQ_S,
             cg * OUT_S:(cg + 1) * OUT_S] = results[c]["out"]
    return full.reshape(B, S, OUT)


def kernel(x, qweight, scales, qzeros, g_idx=None, bias=None, **_unused):
    """Full-input entry point: shards over 8 cores, runs on HW, gathers."""
    from concourse.bass_utils import run_bass_kernel_spmd

    nc = _get_program()
    in_maps = shard_inputs(x, qweight, scales, qzeros, bias)
    res = run_bass_kernel_spmd(nc, in_maps, core_ids=list(range(NCORES)))
    return assemble_output(res.results)


# revision 14
# speedup vs baseline: 1.1740x; 1.0508x over previous
"""GPTQ 4-bit quantized linear (CaiQuantLinear) on 8 TRN2 NeuronCores.

Computes out = x @ dequant(qweight, scales, qzeros) + bias where
  x: (4, 2048, 4096) fp16, qweight: (512, 4096) int32 (8x 4-bit per word,
  packed along input features), scales: (32, 4096) fp16, qzeros: (32, 512)
  int32 (packed along output features), bias: (4096,) fp16.
  Groups are contiguous blocks of 128 input features (g_idx = arange//128).

Sharding: 2x4 grid — 2 sequence-row groups x 4 output-column groups.
Each core gets 4096 seq rows (half of x) and 1024 output columns. This
halves the per-core x DMA-transpose volume (the original bottleneck: the
single HWDGE xbar ring runs at ~150-180 GB/s and starved the PE) while
the dequant work, which runs on the otherwise-idle vector engine, only
doubles. No collectives; the host assembles the 2x4 output grid.

Dequant with ZERO tensor-engine work: qweight is unpacked in its natural
[word-row, out] layout. Nibble s of word-row wr is input feature
k = 8*wr + s, so nibble-plane s of a 128-row word tile is a [128, out]
tile whose partition p holds k = 8*(128r+p) + s. That is a valid matmul
rhs k-tile under a fixed permutation of k — and the contraction is
permutation-invariant as long as lhsT (x^T) rows use the SAME
permutation. The host uploads x with its columns pre-ordered into
(r, s, wr) blocks so DMA-transposed 128-col blocks of x line up with the
nibble planes exactly. Scales/zeros are uploaded with group rows
pre-expanded 16x ([128, ...] layout) so dequant is three flat vector
passes per k-tile (unpack, subtract z+1 with int32->fp16 cast, scale).

Scheduling notes (hard-won from traces):
- Tile serializes every DMA-transpose against every other in-flight DMA
  on any queue (HW deadlock guard), with multi-us round trips. So the
  input loads are issued on the SAME sync queue AHEAD of the transposes
  (strict FIFO -> they complete first, no ping-pong), and only the
  infrequent output stores (scalar/ACT queue) interleave mid-run.
- gpsimd tensor ops slow concurrent vector ops ~3x (SBUF port
  contention) — dequant stays vector-only.
- The vector engine produces dequantized weights at ~1.55 us per
  [128,512] half-tile while the PE consumes k-tiles at 1.73 us (8
  matmuls, the 8-bank psum cap). Dequant emits LEFT halves of all 32
  k-tiles, then RIGHT halves; chunk 0 does two full-chunk k-sweeps (left
  cols for all 8 seq tiles, then right cols), so the PE never outruns
  dequant. Later chunks use 2-seq-tile groups so drains pipeline.
Main loop: 4 chunks of 1024 seq rows; 32 transposes ([1024,128] ->
[128,1024], 256 KB) per chunk; psum tiles are [128, 512] fp32 (1 bank);
drains add bias fp32->fp16 on vector; stores via scalar/ACT HWDGE. The
PE does nothing but the 4096 N=512 matmuls.
"""

import sys

if "/opt/trn_rl_repo" not in sys.path:
    sys.path.insert(0, "/opt/trn_rl_repo")

import numpy as np

B, S, IN, OUT = 4, 2048, 4096, 4096
SEQ = B * S                      # 8192
NCORES = 8
RGRP, CGRP = 2, 4                # core grid: 2 row groups x 4 col groups
SEQ_S = SEQ // RGRP              # 4096 seq rows per core
OUT_S = OUT // CGRP              # 1024 output columns per core
PACK = 8                         # int32 packs 8 nibbles
GSIZE = 128                      # group size == k-tile size
CHUNK = 1024                     # seq rows per transpose chunk
RBLK = IN // (PACK * 128)        # 4 word-row blocks of 128

_CACHE = {}


def _build(seq, out_s, chunk):
    """Build + compile the per-core Bass program. All cores run the same
    NEFF on their own input slices (SPMD, no collectives)."""
    import concourse.bass as bass  # noqa: F401
    import concourse.mybir as mybir
    import concourse.tile as tile
    from concourse import bacc

    dt = mybir.dt
    op = mybir.AluOpType
    P = 128
    KT = IN // P                  # 32 k-tiles (= groups), order (r, s)
    QR = IN // PACK               # 512 qweight word rows
    NCH = seq // chunk            # 4 seq chunks
    ST = chunk // P               # 8 seq tiles per chunk
    NB = out_s // 512             # 2 psum n-blocks (halves) per seq tile
    ZC = out_s // PACK            # 128 qzeros word cols per core
    H = 512                       # psum half width

    nc = bacc.Bacc("TRN2", target_bir_lowering=False, debug=False,
                   num_devices=NCORES)

    x_d = nc.dram_tensor("x", (seq, IN), dt.float16, kind="ExternalInput")
    qw_d = nc.dram_tensor("qweight", (QR, out_s), dt.int32,
                          kind="ExternalInput")
    se_d = nc.dram_tensor("sexp", (P, RBLK * out_s), dt.float16,
                          kind="ExternalInput")
    qz_d = nc.dram_tensor("qzexp", (P, RBLK * ZC), dt.int32,
                          kind="ExternalInput")
    b_d = nc.dram_tensor("bias", (1, out_s), dt.float16, kind="ExternalInput")
    out_d = nc.dram_tensor("out", (seq, out_s), dt.float16,
                           kind="ExternalOutput")

    x = x_d.ap()
    qw = qw_d.ap()
    sexp_in = se_d.ap()
    qzexp_in = qz_d.ap()
    bias = b_d.ap()
    out = out_d.ap()

    with tile.TileContext(nc) as tc:
        with (
            tc.tile_pool(name="const", bufs=1) as const_pool,
            tc.tile_pool(name="w", bufs=1) as w_pool,
            tc.tile_pool(name="qn", bufs=RBLK) as qn_pool,
            tc.tile_pool(name="wq", bufs=2) as wq_pool,
            tc.tile_pool(name="wd", bufs=2) as wd_pool,
            tc.tile_pool(name="xt", bufs=38) as xt_pool,
            tc.tile_pool(name="ot", bufs=4) as out_pool,
            tc.tile_pool(name="ps", bufs=8, space="PSUM") as psum_pool,
        ):
            # ---- input loads: on the SYNC queue, ahead of all transposes
            # (strict FIFO avoids the transpose-vs-DMA serialization) ----
            qz_exp = const_pool.tile([P, RBLK * ZC], dt.int32)
            nc.sync.dma_start(qz_exp, qzexp_in)
            s_exp = const_pool.tile([P, RBLK * out_s], dt.float16)
            nc.sync.dma_start(s_exp, sexp_in)
            q_nats = []
            for r in range(RBLK):
                q_nat = qn_pool.tile([P, out_s], dt.int32, tag="qn")
                nc.sync.dma_start(q_nat, qw[r * P:(r + 1) * P, :])
                q_nats.append(q_nat)
            # bias: scalar queue (off the critical path, needed at drains)
            bias16 = const_pool.tile([P, out_s], dt.float16)
            nc.scalar.dma_start(bias16, bias.to_broadcast((P, out_s)))
            bias32 = const_pool.tile([P, out_s], dt.float32)
            nc.vector.tensor_copy(bias32, bias16)

            # z+1 in expanded [P, (r, n)] layout, int32 (bitVec unpack
            # can't cast; the arithmetic subtract below casts to fp16).
            # Unpacked per r-block so r=0's zeros are ready ASAP and the
            # first dequant isn't gated on the whole z pass.
            z1 = const_pool.tile([P, RBLK * out_s], dt.int32)
            z1v = z1.rearrange("p (r c s) -> p r c s", r=RBLK, s=PACK)
            qzv = qz_exp.rearrange("p (r c) -> p r c", r=RBLK)
            for r in range(RBLK):
                for sz in range(PACK):
                    nc.vector.tensor_scalar(
                        out=z1v[:, r, :, sz], in0=qzv[:, r, :],
                        scalar1=4 * sz, scalar2=0xF,
                        op0=op.logical_shift_right, op1=op.bitwise_and)
                nc.vector.tensor_scalar_add(
                    z1[:, r * out_s:(r + 1) * out_s],
                    z1[:, r * out_s:(r + 1) * out_s], 1)

            # ---- dequant (vector-only), by (half, k-tile) ----
            # w_all[:, j, :]: k-tile j=(r, s); partition p holds input
            # feature k = 8*(128 r + p) + s.
            w_all = w_pool.tile([P, KT, out_s], dt.float16)

            def dequant_half(j, nb):
                r, s = j // PACK, j % PACK
                c0 = nb * H
                wq = wq_pool.tile([P, H], dt.int32, tag="wq")
                nc.vector.tensor_scalar(
                    out=wq, in0=q_nats[r][:, c0:c0 + H], scalar1=4 * s,
                    scalar2=0xF, op0=op.logical_shift_right,
                    op1=op.bitwise_and)
                wd = wd_pool.tile([P, H], dt.float16, tag="wd")
                nc.vector.tensor_tensor(
                    wd, wq, z1[:, r * out_s + c0:r * out_s + c0 + H],
                    op.subtract)
                nc.vector.tensor_tensor(
                    w_all[:, j, c0:c0 + H], wd,
                    s_exp[:, r * out_s + c0:r * out_s + c0 + H], op.mult)

            def drain(pss_st_nb, ch, sts):
                # bias-add psum halves into fp16 out tiles, store rows
                for st in sts:
                    o16 = out_pool.tile([P, out_s], dt.float16, tag="o16")
                    for nb in range(NB):
                        nc.vector.tensor_add(
                            o16[:, nb * H:(nb + 1) * H], pss_st_nb[st][nb],
                            bias32[:, nb * H:(nb + 1) * H])
                    r0 = ch * chunk + st * P
                    nc.scalar.dma_start(out[r0:r0 + P, :], o16)

            def drain_half(ps, ch, st, nb):
                # bias-add one psum half, store the half-row (frees the
                # psum bank without waiting for the other half)
                o16h = out_pool.tile([P, H], dt.float16, tag="o16h")
                nc.vector.tensor_add(o16h, ps, bias32[:, nb * H:(nb + 1) * H])
                r0 = ch * chunk + st * P
                nc.scalar.dma_start(out[r0:r0 + P, nb * H:(nb + 1) * H],
                                    o16h)

            # ---- chunk 0: transposes + left-half dequant + left k-sweep
            # over all 8 seq tiles, then right halves ----
            xts0 = []
            for j in range(KT):
                xtk = xt_pool.tile([P, chunk], dt.float16, tag="xt")
                nc.sync.dma_start(
                    xtk, x[0:chunk, j * P:(j + 1) * P], transpose=True)
                xts0.append(xtk)

            ps0 = {st: [None, None] for st in range(ST)}
            for j in range(KT):
                dequant_half(j, 0)
                for st in range(ST):
                    if j == 0:
                        ps0[st][0] = psum_pool.tile(
                            [P, H], dt.float32, tag="acc", name=f"psA_{st}")
                    nc.tensor.matmul(
                        ps0[st][0], lhsT=xts0[j][:, st * P:(st + 1) * P],
                        rhs=w_all[:, j, 0:H],
                        start=(j == 0), stop=(j == KT - 1))
            for j in range(KT):
                dequant_half(j, 1)
                # interleave the left-sweep drains into the right-half
                # dequant stream so sweep B's psum banks free promptly
                # (vector FIFO: R0, A-drains 0-3, R1, A-drains 4-7, R2...)
                if j in (0, 1):
                    for st in range(4 * j, 4 * j + 4):
                        drain_half(ps0[st][0], 0, st, 0)
                for st in range(ST):
                    if j == 0:
                        ps0[st][1] = psum_pool.tile(
                            [P, H], dt.float32, tag="acc", name=f"psB_{st}")
                    nc.tensor.matmul(
                        ps0[st][1], lhsT=xts0[j][:, st * P:(st + 1) * P],
                        rhs=w_all[:, j, H:out_s],
                        start=(j == 0), stop=(j == KT - 1))
            for st in range(ST):
                drain_half(ps0[st][1], 0, st, 1)

            # ---- chunks 1..3: 2-seq-tile groups, drains pipeline ----
            for ch in range(1, NCH):
                xts = []
                for j in range(KT):
                    xtk = xt_pool.tile([P, chunk], dt.float16, tag="xt")
                    nc.sync.dma_start(
                        xtk,
                        x[ch * chunk:(ch + 1) * chunk, j * P:(j + 1) * P],
                        transpose=True)
                    xts.append(xtk)
                for sg in range(ST // 2):
                    sts = (2 * sg, 2 * sg + 1)
                    pss = {st: [psum_pool.tile([P, H], dt.float32,
                                               tag="acc",
                                               name=f"ps_{ch}_{st}_{nb}")
                                for nb in range(NB)] for st in sts}
                    for j in range(KT):
                        for st in sts:
                            for nb in range(NB):
                                nc.tensor.matmul(
                                    pss[st][nb],
                                    lhsT=xts[j][:, st * P:(st + 1) * P],
                                    rhs=w_all[:, j, nb * H:(nb + 1) * H],
                                    start=(j == 0), stop=(j == KT - 1))
                    drain(pss, ch, sts)

    nc.compile()
    return nc


def _get_program():
    key = (SEQ_S, OUT_S, CHUNK)
    if key not in _CACHE:
        _CACHE[key] = _build(SEQ_S, OUT_S, CHUNK)
    return _CACHE[key]


def _permute_x(x):
    """Reorder x columns into (r, s, wr) blocks: new col (r*8+s)*128 + w
    holds old input feature k = 8*(128 r + w) + s."""
    x2 = np.asarray(x).reshape(SEQ, IN)
    return np.ascontiguousarray(
        x2.reshape(SEQ, RBLK, 128, PACK).transpose(0, 1, 3, 2)
        .reshape(SEQ, IN))


def _expand_groups(a):
    """[32, n] per-group rows -> [128, 4*n]: out[p, r*n + j] =
    a[8*r + p//16, j] (matches the group of k = 8*(128 r + p) + s)."""
    n = a.shape[1]
    e = np.broadcast_to(a.reshape(RBLK, PACK, 1, n), (RBLK, PACK, 16, n))
    return np.ascontiguousarray(
        e.transpose(1, 2, 0, 3).reshape(128, RBLK * n))


def shard_inputs(x, qweight, scales, qzeros, bias):
    """Build the 8 per-core input maps for run_bass_kernel_spmd."""
    xp = _permute_x(x)
    qweight = np.asarray(qweight)
    scales = np.asarray(scales)
    qzeros = np.asarray(qzeros)
    bias = np.asarray(bias)

    in_maps = []
    for c in range(NCORES):
        rg, cg = c // CGRP, c % CGRP
        o0 = cg * OUT_S
        in_maps.append({
            "x": xp[rg * SEQ_S:(rg + 1) * SEQ_S],
            "qweight": np.ascontiguousarray(qweight[:, o0:o0 + OUT_S]),
            "sexp": _expand_groups(scales[:, o0:o0 + OUT_S]),
            "qzexp": _expand_groups(
                qzeros[:, cg * (OUT_S // PACK):(cg + 1) * (OUT_S // PACK)]),
            "bias": np.ascontiguousarray(bias[o0:o0 + OUT_S].reshape(1, -1)),
        })
    return in_maps


def assemble_output(results):
    """Stitch the 2x4 per-core output grid into the full (B, S, OUT)."""
    full = np.empty((SEQ, OUT), dtype=np.float16)
    for c in range(NCORES):
        rg, cg = c // CGRP, c % CGRP
        full[rg * SEQ_S:(rg + 1) * SEQ_S,
             cg * OUT_S:(cg + 1) * OUT_S] = results[c]["out"]
    return full.reshape(B, S, OUT)


def kernel(x, qweight, scales, qzeros, g_idx=None, bias=None, **_unused):
    """Full-input entry point: shards over 8 cores, runs on HW, gathers."""
    from concourse.bass_utils import run_bass_kernel_spmd

    nc = _get_program()
    in_maps = shard_inputs(x, qweight, scales, qzeros, bias)
    res = run_bass_kernel_spmd(nc, in_maps, core_ids=list(range(NCORES)))
    return assemble_output(res.results)
